# revision 1
# baseline (speedup 1.0000x reference)
"""Trainium2 Bass kernel for nn_DependencyParsing (embedding_lookup).

Strategy (pure data-parallel over 8 NeuronCores, B=65536 -> 8192/core):
  - word_table cast to bf16, rows padded to 256B; word embeddings gathered
    feature-major straight into SBUF via SWDGE transpose dma_gather
    (512 idx / instruction HW cap), cycled over 2 SWDGE queues (one
    queue's descriptor ring serializes at ~4.75us/gather; 3+ queues
    corrupt gather packets when HWDGE traffic runs concurrently).
  - pos/dep lookups use no gather at all: pe@Wp + de@Wd is computed as
    onehot @ proj, where proj[s*64+cls] = table_s[cls] @ W_s (built on
    device, 28 small matmuls) and the one-hot [128, 512] per (t) comes
    from a single DVE is_equal of host-replicated fp16 index rows against
    a per-partition iota. Slot s=pos_t on partitions 0..63, dep_t on
    64..127. The combined bias (bw+bp+bd) rides a constant-1 row (t=0,
    partition 63) with proj row 63 = bias.
  - h = x @ W as bf16 matmuls (14 K-blocks x 6 M-tiles per 512 chunk)
    accumulating f32 in PSUM; h^3 = Square(h)*h on ACT+DVE -> bf16.
  - logits = h3 @ Wo; softmax stays class-major (no transpose): ACT
    Exp(logits+bo) from PSUM (logits are tiny, so no max subtraction),
    partition-sum via a ones-vector matmul, DVE reciprocal, PE ones
    broadcast, DVE multiply. Output lands [93, B] and the host transposes.
  - The previous chunk's logits/sum/broadcast matmuls are interleaved
    between the current chunk's M-tiles so the PE never idles long enough
    for the HAM clock gate to re-throttle.
"""

import os

import numpy as np
import ml_dtypes

import concourse.bacc as bacc
import concourse.mybir as mybir
import concourse.tile as tile
from concourse.tile import add_dep_helper
from concourse.bass_utils import run_bass_kernel_spmd

B, T, D, H, V, NPOS, NDEP, OUT = 65536, 7, 100, 700, 32000, 50, 45, 93
NCORES = 8
B_CORE = B // NCORES
CHUNK = 512
P = 128
# M-tiles over the 700 output features of h
MT = [(0, 128), (128, 128), (256, 128), (384, 128), (512, 128), (640, 60)]
# K-blocks for logits: 700 h-features in 6 blocks of 128 (last 60)
LKB = [(0, 128), (128, 128), (256, 128), (384, 128), (512, 128), (640, 60)]
dt = mybir.dt
bf16 = ml_dtypes.bfloat16
NQ = int(os.environ.get("KERNEL_NQ", "2"))

_NC_CACHE = {}


def build_nc(b_core):
    n_chunks = b_core // CHUNK
    nc = bacc.Bacc(None, target_bir_lowering=False, num_swdge_queues=max(NQ, 2))
    with tile.TileContext(nc) as tc:
        with tc.tile_pool(name="dram", bufs=1, space="DRAM") as dram:
            word_tab = dram.tile([V + 1, 128], dt.bfloat16, kind="ExternalInput",
                                 name="word_tab", uniquify=False)
            widx_d = dram.tile([P, T * n_chunks * 32], dt.int16, kind="ExternalInput",
                               name="widx", uniquify=False)
            vidx_d = dram.tile([P, n_chunks * T * CHUNK], dt.float16,
                               kind="ExternalInput", name="vidx", uniquify=False)
            tabt_d = dram.tile([P, 14 * 64], dt.bfloat16, kind="ExternalInput",
                               name="tabT", uniquify=False)
            iota_d = dram.tile([P, 1], dt.float32, kind="ExternalInput",
                               name="iota64", uniquify=False)
            ww_d = dram.tile([P, T * H], dt.bfloat16, kind="ExternalInput",
                             name="w_word", uniquify=False)
            wpd_d = dram.tile([P, 2 * T * H], dt.bfloat16, kind="ExternalInput",
                              name="w_pd", uniquify=False)
            wo_d = dram.tile([P, 6 * 96], dt.bfloat16, kind="ExternalInput",
                             name="w_o", uniquify=False)
            bias_d = dram.tile([1, H], dt.bfloat16, kind="ExternalInput",
                               name="bias_row", uniquify=False)
            bo_d = dram.tile([P, 1], dt.float32, kind="ExternalInput",
                             name="bo_pad", uniquify=False)
            out_d = dram.tile([OUT, b_core], dt.float32, kind="ExternalOutput",
                              name="out", uniquify=False)

            with (
                tc.tile_pool(name="const", bufs=1) as const,
                tc.tile_pool(name="wg", bufs=3) as wg_pool,
                tc.tile_pool(name="vx", bufs=3) as vx_pool,
                tc.tile_pool(name="oh", bufs=3) as oh_pool,
                tc.tile_pool(name="sq", bufs=3) as sq_pool,
                tc.tile_pool(name="h3", bufs=3) as h3_pool,
                tc.tile_pool(name="exq", bufs=3) as ex_pool,
                tc.tile_pool(name="rcq", bufs=2) as rc_pool,
                tc.tile_pool(name="opq", bufs=2) as op_pool,
                tc.tile_pool(name="hps", bufs=1, space="PSUM") as hps_pool,
                tc.tile_pool(name="ltps", bufs=2, space="PSUM") as ltps_pool,
            ):
                preloads = []
                ww_sb = const.tile([P, T * H], dt.bfloat16, name="ww_sb")
                preloads.append(nc.sync.dma_start(out=ww_sb[:], in_=ww_d[:]))
                wpd_sb = const.tile([P, 2 * T * H], dt.bfloat16, name="wpd_sb")
                preloads.append(nc.sync.dma_start(out=wpd_sb[:], in_=wpd_d[:]))
                wo_sb = const.tile([P, 6 * 96], dt.bfloat16, name="wo_sb")
                preloads.append(nc.sync.dma_start(out=wo_sb[:], in_=wo_d[:]))
                widx_sb = const.tile([P, T * n_chunks * 32], dt.int16, name="widx_sb")
                preloads.append(nc.sync.dma_start(out=widx_sb[:], in_=widx_d[:]))
                tabt_sb = const.tile([P, 14 * 64], dt.bfloat16, name="tabt_sb")
                preloads.append(nc.sync.dma_start(out=tabt_sb[:], in_=tabt_d[:]))
                iota_sb = const.tile([P, 1], dt.float32, name="iota_sb")
                preloads.append(nc.sync.dma_start(out=iota_sb[:], in_=iota_d[:]))
                bo_sb = const.tile([P, 1], dt.float32, name="bo_sb")
                preloads.append(nc.sync.dma_start(out=bo_sb[:], in_=bo_d[:]))
                ones_col = const.tile([P, 1], dt.float32, name="ones_col")
                nc.vector.memset(ones_col[:, :], 1.0)
                ones_row = const.tile([1, 96], dt.float32, name="ones_row")
                nc.vector.memset(ones_row[:, :], 1.0)

                # ---- build proj[s*64+cls] = table_s[cls] @ W_s on device ----
                proj_sb = const.tile([P, T * H], dt.bfloat16, name="proj_sb")
                for t in range(T):
                    pp1 = ltps_pool.tile([P, 512], dt.float32, name="pp1", tag="lt")
                    pp2 = ltps_pool.tile([P, 188], dt.float32, name="pp2", tag="lt")
                    for half in range(2):
                        s = t * 2 + half
                        lhsT = tabt_sb[:, s * 64:(s + 1) * 64]
                        nc.tensor.matmul(pp1[64 * half:64 * half + 64, :], lhsT,
                                         wpd_sb[:, s * H:s * H + 512],
                                         start=True, stop=True)
                        nc.tensor.matmul(pp2[64 * half:64 * half + 64, :], lhsT,
                                         wpd_sb[:, s * H + 512:s * H + 700],
                                         start=True, stop=True)
                    nc.scalar.activation(proj_sb[:, t * H:t * H + 512], pp1[:, :],
                                         mybir.ActivationFunctionType.Copy)
                    nc.scalar.activation(proj_sb[:, t * H + 512:t * H + 700], pp2[:, :],
                                         mybir.ActivationFunctionType.Copy)
                # combined bias rides one-hot row 63 of tile t=0
                preloads.append(
                    nc.sync.dma_start(out=proj_sb[63:64, 0:H], in_=bias_d[:, :]))

                # Deferred epilogue pieces for the previous chunk.
                pend = {}

                def emit_logits(h3_list):
                    lg_ps = ltps_pool.tile([P, CHUNK], dt.float32, name="lg_ps", tag="lt")
                    for j, (k0, ksz) in enumerate(LKB):
                        nc.tensor.matmul(
                            lg_ps[:96, :],
                            wo_sb[:ksz, j * 96:(j + 1) * 96],
                            h3_list[j][:ksz, :],
                            start=(j == 0), stop=(j == 5),
                        )
                    ex = ex_pool.tile([P, CHUNK], dt.float32, name="ex")
                    nc.scalar.activation(ex[:96, :], lg_ps[:96, :],
                                         mybir.ActivationFunctionType.Exp,
                                         bias=bo_sb[:96, :])
                    pend["ex"] = ex

                def emit_sum():
                    sum_ps = ltps_pool.tile([P, CHUNK], dt.float32, name="sum_ps", tag="lt")
                    nc.tensor.matmul(sum_ps[:1, :], ones_col[:OUT, :],
                                     pend["ex"][:OUT, :], start=True, stop=True)
                    rc = rc_pool.tile([1, CHUNK], dt.float32, name="rc")
                    nc.vector.reciprocal(rc[:1, :], sum_ps[:1, :])
                    pend["rc"] = rc

                def emit_bcast(cc):
                    rcb_ps = ltps_pool.tile([P, CHUNK], dt.float32, name="rcb_ps", tag="lt")
                    nc.tensor.matmul(rcb_ps[:96, :], ones_row[:1, :96],
                                     pend["rc"][:1, :], start=True, stop=True)
                    opt = op_pool.tile([P, CHUNK], dt.float32, name="opt")
                    nc.vector.tensor_mul(opt[:96, :], pend["ex"][:96, :],
                                         rcb_ps[:96, :])
                    nc.sync.dma_start(out=out_d[:, cc * CHUNK:(cc + 1) * CHUNK],
                                      in_=opt[:OUT, :])

                qn = 0
                prev_h3 = None
                for c in range(n_chunks):
                    # ---- word gathers (feature-major), cycled over queues ----
                    wg = []
                    for t in range(T):
                        g = wg_pool.tile([P, CHUNK], dt.bfloat16, name=f"wg{t}")
                        gi = nc.gpsimd.dma_gather(
                            g.rearrange("p (o n) -> p o n", o=1),
                            word_tab[:],
                            widx_sb[:, (t * n_chunks + c) * 32:(t * n_chunks + c + 1) * 32],
                            CHUNK, CHUNK, 128, transpose=True, queue_num=qn % NQ,
                        )
                        if c == 0:
                            # keep transpose-gather traffic strictly after the
                            # preload DMAs (concurrent HWDGE transfers have
                            # been observed to corrupt gather/preload packets)
                            for pl in preloads:
                                add_dep_helper(gi.ins, pl.ins)
                        qn += 1
                        wg.append(g)

                    # ---- pos/dep one-hots from replicated fp16 idx rows ----
                    vx = vx_pool.tile([P, T * CHUNK], dt.float16, name="vx")
                    nc.sync.dma_start(
                        out=vx[:], in_=vidx_d[:, c * T * CHUNK:(c + 1) * T * CHUNK])
                    oh = []
                    for t in range(T):
                        o = oh_pool.tile([P, CHUNK], dt.bfloat16, name=f"oh{t}")
                        nc.vector.tensor_scalar(
                            o[:, :], vx[:, t * CHUNK:(t + 1) * CHUNK],
                            iota_sb[:, :], None, mybir.AluOpType.is_equal)
                        oh.append(o)

                    # ---- h = x @ W (+bias via one-hot row), h3 = h^2 * h ----
                    # Previous chunk's epilogue matmuls are interleaved between
                    # M-tiles so their cross-engine deps resolve off PE time.
                    h3 = []
                    for mi, (m0, msz) in enumerate(MT):
                        hp = hps_pool.tile([P, CHUNK], dt.float32, name=f"hps{mi}")
                        kb = 0
                        for t in range(T):
                            nc.tensor.matmul(
                                hp[:msz, :],
                                ww_sb[:, t * H + m0: t * H + m0 + msz],
                                wg[t][:, :],
                                start=(kb == 0), stop=(kb == 13),
                            )
                            kb += 1
                        for t in range(T):
                            nc.tensor.matmul(
                                hp[:msz, :],
                                proj_sb[:, t * H + m0: t * H + m0 + msz],
                                oh[t][:, :],
                                start=(kb == 0), stop=(kb == 13),
                            )
                            kb += 1
                        sq = sq_pool.tile([P, CHUNK], dt.float32, name="sq")
                        nc.scalar.square(sq[:msz, :], hp[:msz, :])
                        h3t = h3_pool.tile([P, CHUNK], dt.bfloat16, name=f"h3_{mi}")
                        nc.vector.tensor_mul(h3t[:msz, :], sq[:msz, :], hp[:msz, :])
                        h3.append(h3t)
                        if prev_h3 is not None:
                            if mi == 0:
                                emit_logits(prev_h3)
                            elif mi == 2:
                                emit_sum()
                            elif mi == 4:
                                emit_bcast(c - 1)
                    prev_h3 = h3

                # tail epilogue for the last chunk
                emit_logits(prev_h3)
                emit_sum()
                emit_bcast(n_chunks - 1)
    nc.compile()
    return nc


def _wrap_idx(idx_tc):
    """[CHUNK] -> [128, 32] wrapped (i -> [i%16, i//16]) + replicated x8."""
    n = idx_tc.shape[0]
    w = idx_tc.reshape(n // 16, 16).T  # [16, n/16]
    return np.tile(w, (8, 1))


def prep_inputs(word_idx, pos_idx, dep_idx, word_table, pos_table, dep_table,
                Ww, bw, Wp, bp, Wd, bd, Wo, bo, b_core):
    """Returns (shared_map, per_core_fn). Host work is layout-only + small."""
    n_chunks = b_core // CHUNK

    wt = np.zeros((V + 1, 128), dtype=bf16)
    wt[:V, :D] = np.asarray(word_table, np.float32).astype(bf16)

    # pos/dep tables transposed: tabT[p, s*64+cls] = table_s[cls, p]
    tabt = np.zeros((P, 14 * 64), dtype=bf16)
    pt = np.asarray(pos_table, np.float32).astype(bf16)
    dtab = np.asarray(dep_table, np.float32).astype(bf16)
    for t in range(T):
        tabt[:D, (2 * t) * 64:(2 * t) * 64 + NPOS] = pt.T
        tabt[:D, (2 * t + 1) * 64:(2 * t + 1) * 64 + NDEP] = dtab.T

    iota64 = (np.arange(P) % 64).astype(np.float32).reshape(P, 1)

    bias_all = (np.asarray(bw, np.float32) + np.asarray(bp, np.float32)
                + np.asarray(bd, np.float32))
    bias_row = bias_all.astype(bf16).reshape(1, H)

    def pack_w(Wmat):
        arr = np.zeros((T, P, H), dtype=bf16)
        Wmat = np.asarray(Wmat, np.float32)
        for t in range(T):
            arr[t, :D, :] = Wmat[D * t:D * (t + 1), :].astype(bf16)
        return arr

    ww = pack_w(Ww)
    wp = pack_w(Wp)
    wd = pack_w(Wd)
    wpd = np.zeros((T, 2, P, H), dtype=bf16)
    wpd[:, 0] = wp
    wpd[:, 1] = wd

    wo = np.zeros((6, P, 96), dtype=bf16)
    Wo32 = np.asarray(Wo, np.float32)
    for j, (k0, ksz) in enumerate(LKB):
        wo[j, :ksz, :OUT] = Wo32[k0:k0 + ksz, :].astype(bf16)

    bo_pad = np.zeros((P, 1), dtype=np.float32)
    bo_pad[:OUT, 0] = np.asarray(bo, np.float32)

    shared = {
        "word_tab": wt,
        "tabT": tabt,
        "iota64": iota64,
        "bias_row": bias_row,
        "w_word": np.ascontiguousarray(ww.transpose(1, 0, 2)).reshape(P, T * H),
        "w_pd": np.ascontiguousarray(wpd.transpose(2, 0, 1, 3)).reshape(P, 2 * T * H),
        "w_o": np.ascontiguousarray(wo.transpose(1, 0, 2)).reshape(P, 6 * 96),
        "bo_pad": bo_pad,
    }

    wi = np.asarray(word_idx, np.int64).copy()
    wi[wi < 0] = V
    wi = wi.astype(np.int16)
    pi16 = np.asarray(pos_idx, np.int32).astype(np.float16)
    di16 = np.asarray(dep_idx, np.int32).astype(np.float16)

    def core_map(core):
        s = slice(core * b_core, (core + 1) * b_core)
        wic = wi[s]
        widx = np.zeros((P, T, n_chunks, 32), dtype=np.int16)
        for t in range(T):
            for c in range(n_chunks):
                widx[:, t, c, :] = _wrap_idx(wic[c * CHUNK:(c + 1) * CHUNK, t])

        # vidx[p, c, t, i]: p<64 -> pos_idx, p>=64 -> dep_idx; (t=0, p=63) = 63
        pc = pi16[s].reshape(n_chunks, CHUNK, T).transpose(0, 2, 1)
        dc = di16[s].reshape(n_chunks, CHUNK, T).transpose(0, 2, 1)
        vidx = np.empty((P, n_chunks, T, CHUNK), dtype=np.float16)
        vidx[:64] = pc[None, :, :, :]
        vidx[64:] = dc[None, :, :, :]
        vidx[63, :, 0, :] = np.float16(63.0)

        m = dict(shared)
        m["widx"] = widx.reshape(P, T * n_chunks * 32)
        m["vidx"] = np.ascontiguousarray(vidx).reshape(P, n_chunks * T * CHUNK)
        return m

    return shared, core_map


def kernel(**inputs):
    b_core = B_CORE
    if b_core not in _NC_CACHE:
        _NC_CACHE[b_core] = build_nc(b_core)
    nc = _NC_CACHE[b_core]

    _, core_map = prep_inputs(b_core=b_core, **inputs)
    in_maps = [core_map(i) for i in range(NCORES)]
    res = run_bass_kernel_spmd(nc, in_maps, core_ids=list(range(NCORES)))
    out = np.concatenate([r["out"] for r in res.results], axis=1)  # [93, B]
    return np.ascontiguousarray(out.T).astype(np.float32)



# revision 9
# speedup vs baseline: 2.1576x; 2.1576x over previous
"""Trainium2 Bass kernel for nn_DependencyParsing (embedding_lookup).

Strategy (pure data-parallel over 8 NeuronCores, B=65536 -> 8192/core):
  - Everything on the PE runs as fp8(e4m3) DoubleRow matmuls: each
    instruction contracts 256 K-slots (128 partitions x 2 byte-planes)
    at the same per-instruction cost as bf16 (~226ns @ N=512).
  - word_table rows are 256B of packed fp8 feature-pairs (x16 scale):
    feature f of a row lands at SBUF (partition f//2, byte f%2) via the
    same u16 transpose dma_gather as a bf16 table would use. 4 SWDGE
    queues sustain ~1.27us/gather (112 gathers -> ~142us), overlapped
    with compute. Gathers also zero partitions 50..127 (table row tail).
  - pos/dep one-hots are precomputed on the host as fp8 pairs and DMA'd
    into partitions 50..98 of the *same* gathered tiles (after the
    gathers; HWDGE concurrent with SWDGE verified clean). The matching
    rows of the combined weight tensor hold host-computed
    proj = table_s @ W_s (x256) and the summed bias (x256) rides a
    constant-one slot at (t=0, partition 98). So h = x @ W is just
    7 DoubleRow matmuls per M-tile: 42 + 3 (logits) = 45 PE instrs per
    512-sample chunk.
  - h (PSUM, x256) -> ACT Square(scale 2^-8) -> DVE mul -> h3 fp8 (x256)
    packed [128, 3, 2, 512] for 3 DoubleRow logits matmuls (Wo x16).
  - ACT Exp(lg x 2^-12 + bo) -> ex bf16 [93, 512] -> DMA out per chunk.
    Softmax normalization (divide by row sum) happens on the host.
  - The previous chunk's logits/exp/out are interleaved between the
    current chunk's M-tiles so the PE never idles long enough for the
    HAM clock gate to re-throttle.
"""

import os

import numpy as np
import ml_dtypes

import concourse.bacc as bacc
import concourse.mybir as mybir
import concourse.tile as tile
from concourse.tile import add_dep_helper
from concourse.bass_utils import run_bass_kernel_spmd

B, T, D, H, V, NPOS, NDEP, OUT = 65536, 7, 100, 700, 32000, 50, 45, 93
NCORES = 8
B_CORE = B // NCORES
CHUNK = 512
P = 128
MT = [(0, 128), (128, 128), (256, 128), (384, 128), (512, 128), (640, 64)]
MOFF = [0, 256, 512, 768, 1024, 1280]  # 2*msz-prefix offsets within a t-block
TBLK = 1408  # 2 * sum(msz) per token
dt = mybir.dt
f8 = ml_dtypes.float8_e4m3
bf16 = ml_dtypes.bfloat16
NQ = int(os.environ.get("KERNEL_NQ", "4"))
DR = mybir.MatmulPerfMode.DoubleRow
F8_ONE = np.float32(1.0).astype(f8).view(np.uint8)  # 0x38

_NC_CACHE = {}


def build_nc(b_core):
    n_chunks = b_core // CHUNK
    nc = bacc.Bacc(None, target_bir_lowering=False, num_swdge_queues=NQ)
    with tile.TileContext(nc) as tc:
        with tc.tile_pool(name="dram", bufs=1, space="DRAM") as dram:
            word_tab = dram.tile([V + 1, 128], dt.uint16, kind="ExternalInput",
                                 name="word_tab", uniquify=False)
            widx_d = dram.tile([P, T * n_chunks * 32], dt.int16,
                               kind="ExternalInput", name="widx", uniquify=False)
            ohx_d = dram.tile([49, n_chunks * T * CHUNK], dt.uint16,
                              kind="ExternalInput", name="ohx", uniquify=False)
            wcomb_d = dram.tile([P, T * TBLK], dt.float8e4,
                                kind="ExternalInput", name="wcomb", uniquify=False)
            wo_d = dram.tile([P, 3 * 2 * 96], dt.float8e4,
                             kind="ExternalInput", name="w_o", uniquify=False)
            bo_d = dram.tile([P, 1], dt.float32, kind="ExternalInput",
                             name="bo_pad", uniquify=False)
            out_d = dram.tile([OUT, b_core], dt.bfloat16, kind="ExternalOutput",
                              name="out", uniquify=False)

            with (
                tc.tile_pool(name="const", bufs=1) as const,
                tc.tile_pool(name="wt", bufs=3) as wt_pool,
                tc.tile_pool(name="sq", bufs=3) as sq_pool,
                tc.tile_pool(name="h3", bufs=3) as h3_pool,
                tc.tile_pool(name="exq", bufs=3) as ex_pool,
                tc.tile_pool(name="hps", bufs=1, space="PSUM") as hps_pool,
                tc.tile_pool(name="ltps", bufs=2, space="PSUM") as ltps_pool,
            ):
                preloads = []
                wcomb_sb = const.tile([P, T * TBLK], dt.float8e4, name="wcomb_sb")
                preloads.append(nc.sync.dma_start(out=wcomb_sb[:], in_=wcomb_d[:]))
                wo_sb = const.tile([P, 3 * 2 * 96], dt.float8e4, name="wo_sb")
                preloads.append(nc.sync.dma_start(out=wo_sb[:], in_=wo_d[:]))
                widx_sb = const.tile([P, T * n_chunks * 32], dt.int16,
                                     name="widx_sb")
                preloads.append(nc.sync.dma_start(out=widx_sb[:], in_=widx_d[:]))
                bo_sb = const.tile([P, 1], dt.float32, name="bo_sb")
                preloads.append(nc.sync.dma_start(out=bo_sb[:], in_=bo_d[:]))

                wo_v = wo_sb.rearrange("p (r j m) -> p r j m", r=3, j=2)

                def wslice(t, mi, msz):
                    o = t * TBLK + MOFF[mi]
                    return wcomb_sb[:, o:o + 2 * msz].rearrange(
                        "p (j m) -> p j m", j=2)

                # zero the h3 tail (plane 5, partitions 60..127) once per
                # physical buffer: stale SBUF bytes there could decode as
                # fp8 NaN and poison the logits accumulation.
                h3_warm = []
                for _ in range(3):
                    hw = h3_pool.tile([P, 3, 2, CHUNK], dt.float8e4, name="h3")
                    nc.vector.memset(hw[:, 2, 1, :], 0.0)
                    h3_warm.append(hw)

                pend = {}

                def emit_logits(h3_prev):
                    lg = ltps_pool.tile([P, CHUNK], dt.float32, name="lg",
                                        tag="lt")
                    for pr in range(3):
                        nc.tensor.matmul(
                            lg[:96, :], wo_v[:, pr, :, :], h3_prev[:, pr, :, :],
                            start=(pr == 0), stop=(pr == 2), perf_mode=DR)
                    pend["lg"] = lg

                def emit_ex():
                    ex = ex_pool.tile([96, CHUNK], dt.bfloat16, name="ex")
                    nc.scalar.activation(ex[:96, :], pend["lg"][:96, :],
                                         mybir.ActivationFunctionType.Exp,
                                         bias=bo_sb[:96, :], scale=1.0 / 4096)
                    pend["ex"] = ex

                def emit_out(cc):
                    nc.sync.dma_start(out=out_d[:, cc * CHUNK:(cc + 1) * CHUNK],
                                      in_=pend["ex"][:OUT, :])

                qn = 0
                prev_h3 = None
                for c in range(n_chunks):
                    wt = wt_pool.tile([P, T, CHUNK], dt.uint16, name="wt")
                    for t in range(T):
                        gi = nc.gpsimd.dma_gather(
                            wt[:, t:t + 1, :], word_tab[:],
                            widx_sb[:, (t * n_chunks + c) * 32:
                                    (t * n_chunks + c + 1) * 32],
                            CHUNK, CHUNK, 128, transpose=True,
                            queue_num=qn % NQ,
                        )
                        if c == 0:
                            for pl in preloads:
                                add_dep_helper(gi.ins, pl.ins)
                        qn += 1
                    # host one-hots (pos/dep/bias) overwrite partitions
                    # 50..98 of every token's gathered tile
                    nc.sync.dma_start(
                        out=wt[50:99, :, :].rearrange("p a b -> p (a b)"),
                        in_=ohx_d[:, c * T * CHUNK:(c + 1) * T * CHUNK])

                    h3t = h3_pool.tile([P, 3, 2, CHUNK], dt.float8e4, name="h3")
                    for mi, (m0, msz) in enumerate(MT):
                        hp = hps_pool.tile([P, CHUNK], dt.float32,
                                           name=f"hps{mi}")
                        for t in range(T):
                            rhs = wt[:, t, :].bitcast(dt.float8e4).rearrange(
                                "p (n j) -> p j n", j=2)
                            nc.tensor.matmul(
                                hp[:msz, :], wslice(t, mi, msz),
                                rhs, start=(t == 0), stop=(t == T - 1),
                                perf_mode=DR)
                        sq = sq_pool.tile([P, CHUNK], dt.float32, name="sq")
                        nc.scalar.activation(sq[:msz, :], hp[:msz, :],
                                             mybir.ActivationFunctionType.Square,
                                             scale=1.0 / 256)
                        nc.vector.tensor_mul(h3t[:msz, mi // 2, mi % 2, :],
                                             sq[:msz, :], hp[:msz, :])
                        if prev_h3 is not None:
                            if mi == 0:
                                emit_logits(prev_h3)
                            elif mi == 2:
                                emit_ex()
                            elif mi == 4:
                                emit_out(c - 1)
                    prev_h3 = h3t

                emit_logits(prev_h3)
                emit_ex()
                emit_out(n_chunks - 1)
    nc.compile()
    return nc


def _wrap_idx(idx_tc):
    """[CHUNK] -> [128, 32] wrapped (i -> [i%16, i//16]) + replicated x8."""
    n = idx_tc.shape[0]
    w = idx_tc.reshape(n // 16, 16).T
    return np.tile(w, (8, 1))


def prep_inputs(word_idx, pos_idx, dep_idx, word_table, pos_table, dep_table,
                Ww, bw, Wp, bp, Wd, bd, Wo, bo, b_core):
    """Returns (shared_map, per_core_fn)."""
    n_chunks = b_core // CHUNK

    # ---- word table: 256B rows of fp8 pairs (x16) ----
    wt8 = np.zeros((V + 1, 256), dtype=np.uint8)
    wt8[:V, :D] = (np.asarray(word_table, np.float32) * 16).astype(f8).view(
        np.uint8)
    wt_q = wt8.view(np.uint16)  # [V+1, 128]

    # ---- combined weights [128, T, 2, H] fp8 ----
    Ww32 = np.asarray(Ww, np.float32)
    Wp32 = np.asarray(Wp, np.float32)
    Wd32 = np.asarray(Wd, np.float32)
    pt32 = np.asarray(pos_table, np.float32)
    dt32 = np.asarray(dep_table, np.float32)
    bias_all = (np.asarray(bw, np.float32) + np.asarray(bp, np.float32)
                + np.asarray(bd, np.float32))

    wfull = np.zeros((P, T, 2, H), dtype=f8)
    for t in range(T):
        wq = (Ww32[D * t:D * (t + 1)] * 16).astype(f8)        # [100, H]
        f = np.arange(D)
        wfull[f // 2, t, f % 2, :] = wq
        projp = (pt32 @ Wp32[D * t:D * (t + 1)] * 256).astype(f8)  # [50, H]
        s = np.arange(NPOS)
        wfull[50 + s // 2, t, s % 2, :] = projp
        projd = (dt32 @ Wd32[D * t:D * (t + 1)] * 256).astype(f8)  # [45, H]
        s = np.arange(NDEP)
        wfull[75 + s // 2, t, s % 2, :] = projd
    wfull[98, 0, 0, :] = (bias_all * 256).astype(f8)
    # repack into contiguous per-(t, M-tile) [2, msz] blocks
    wcomb_host = np.zeros((P, T * TBLK), dtype=f8)
    for t in range(T):
        for mi, (m0, msz) in enumerate(MT):
            mhi = min(m0 + msz, H)
            blk = np.zeros((P, 2, msz), dtype=f8)
            blk[:, :, :mhi - m0] = wfull[:, t, :, m0:mhi]
            o = t * TBLK + MOFF[mi]
            wcomb_host[:, o:o + 2 * msz] = blk.reshape(P, 2 * msz)

    # ---- logits weights [128, 3, 2, 96] fp8: slot (p,pr,j) = h-feat
    # 128*(2pr+j)+p ----
    Wo32 = np.asarray(Wo, np.float32)
    wo = np.zeros((P, 3, 2, 96), dtype=f8)
    for pr in range(3):
        for j in range(2):
            mi = 2 * pr + j
            m0, msz = MT[mi]
            mhi = min(m0 + msz, H)
            wo[:mhi - m0, pr, j, :OUT] = (Wo32[m0:mhi] * 16).astype(f8)
    wo_host = np.ascontiguousarray(wo).reshape(P, 3 * 2 * 96)

    bo_pad = np.zeros((P, 1), dtype=np.float32)
    bo_pad[:OUT, 0] = np.asarray(bo, np.float32)

    shared = {
        "word_tab": wt_q,
        "wcomb": wcomb_host,
        "w_o": wo_host,
        "bo_pad": bo_pad,
    }

    wi = np.asarray(word_idx, np.int64).copy()
    wi[wi < 0] = V
    wi = wi.astype(np.int16)
    pi = np.asarray(pos_idx, np.int32)
    di = np.asarray(dep_idx, np.int32)

    def core_map(core):
        s = slice(core * b_core, (core + 1) * b_core)
        wic = wi[s]
        widx = np.zeros((P, T, n_chunks, 32), dtype=np.int16)
        for t in range(T):
            for c in range(n_chunks):
                widx[:, t, c, :] = _wrap_idx(wic[c * CHUNK:(c + 1) * CHUNK, t])

        # one-hot pairs: [49, 2, n_chunks, T, CHUNK] u8 -> u16 [49, ...]
        oh = np.zeros((49, 2, n_chunks, T, CHUNK), dtype=np.uint8)
        pc = pi[s].reshape(n_chunks, CHUNK, T)
        dc = di[s].reshape(n_chunks, CHUNK, T)
        cg, ig, tg = np.meshgrid(np.arange(n_chunks), np.arange(CHUNK),
                                 np.arange(T), indexing="ij")
        oh[pc // 2, pc % 2, cg, tg, ig] = F8_ONE
        oh[25 + dc // 2, dc % 2, cg, tg, ig] = F8_ONE
        oh[48, 0, :, 0, :] = F8_ONE  # bias rides token 0
        ohx = np.ascontiguousarray(oh.transpose(0, 2, 3, 4, 1)).view(
            np.uint16).reshape(49, n_chunks * T * CHUNK)

        m = dict(shared)
        m["widx"] = widx.reshape(P, T * n_chunks * 32)
        m["ohx"] = ohx
        return m

    return shared, core_map


def kernel(**inputs):
    b_core = B_CORE
    if b_core not in _NC_CACHE:
        _NC_CACHE[b_core] = build_nc(b_core)
    nc = _NC_CACHE[b_core]

    _, core_map = prep_inputs(b_core=b_core, **inputs)
    in_maps = [core_map(i) for i in range(NCORES)]
    res = run_bass_kernel_spmd(nc, in_maps, core_ids=list(range(NCORES)))
    ex = np.concatenate([r["out"] for r in res.results], axis=1)  # [93, B]
    ex = np.ascontiguousarray(ex.T).astype(np.float32)            # [B, 93]
    return ex / ex.sum(axis=1, keepdims=True)


# revision 13
# speedup vs baseline: 2.3110x; 1.0711x over previous
"""Trainium2 Bass kernel for nn_DependencyParsing (embedding_lookup).

Strategy (pure data-parallel over 8 NeuronCores, B=65536 -> 8192/core):
  - Everything on the PE runs as fp8(e4m3) DoubleRow matmuls: each
    instruction contracts 256 K-slots (128 partitions x 2 byte-planes)
    at the same per-instruction cost as bf16 (~226ns @ N=512).
  - word_table rows are 256B of packed fp8 feature-pairs (x16 scale):
    feature f of a row lands at SBUF (partition f//2, byte f%2) via the
    same u16 transpose dma_gather as a bf16 table would use. 4 SWDGE
    queues sustain ~1.27us/gather (112 gathers -> ~142us), overlapped
    with compute. Gathers also zero partitions 50..127 (table row tail).
  - pos/dep one-hots are precomputed on the host as fp8 pairs and DMA'd
    into partitions 50..98 of the *same* gathered tiles (after the
    gathers; HWDGE concurrent with SWDGE verified clean). The matching
    rows of the combined weight tensor hold host-computed
    proj = table_s @ W_s (x256) and the summed bias (x256) rides a
    constant-one slot at (t=0, partition 98). So h = x @ W is just
    7 DoubleRow matmuls per M-tile: 42 + 3 (logits) = 45 PE instrs per
    512-sample chunk.
  - h (PSUM, x256) -> ACT Square(scale 2^-8) -> DVE mul -> h3 fp8 (x256)
    packed [128, 3, 2, 512] for 3 DoubleRow logits matmuls (Wo x16).
  - ACT Exp(lg x 2^-12 + bo) -> ex bf16 [93, 512] -> DMA out per chunk.
    Softmax normalization (divide by row sum) happens on the host.
  - The previous chunk's logits/exp/out are interleaved between the
    current chunk's M-tiles so the PE never idles long enough for the
    HAM clock gate to re-throttle.
"""

import os

import numpy as np
import ml_dtypes

import concourse.bacc as bacc
import concourse.mybir as mybir
import concourse.tile as tile
from concourse.tile import add_dep_helper
from concourse.bass_utils import run_bass_kernel_spmd

B, T, D, H, V, NPOS, NDEP, OUT = 65536, 7, 100, 700, 32000, 50, 45, 93
NCORES = 8
B_CORE = B // NCORES
CHUNK = 512
P = 128
MT = [(0, 128), (128, 128), (256, 128), (384, 128), (512, 128), (640, 64)]
MOFF = [0, 256, 512, 768, 1024, 1280]  # 2*msz-prefix offsets within a t-block
TBLK = 1408  # 2 * sum(msz) per token
dt = mybir.dt
f8 = ml_dtypes.float8_e4m3
bf16 = ml_dtypes.bfloat16
NQ = int(os.environ.get("KERNEL_NQ", "4"))
DR = mybir.MatmulPerfMode.DoubleRow
F8_ONE = np.float32(1.0).astype(f8).view(np.uint8)  # 0x38

_NC_CACHE = {}


def build_nc(b_core):
    n_chunks = b_core // CHUNK
    nc = bacc.Bacc(None, target_bir_lowering=False, num_swdge_queues=NQ)
    with tile.TileContext(nc) as tc:
        with tc.tile_pool(name="dram", bufs=1, space="DRAM") as dram:
            word_tab = dram.tile([V + 1, 128], dt.uint16, kind="ExternalInput",
                                 name="word_tab", uniquify=False)
            widx_d = dram.tile([P, T * n_chunks * 32], dt.int16,
                               kind="ExternalInput", name="widx", uniquify=False)
            ohx_d = dram.tile([49, n_chunks * T * CHUNK], dt.uint16,
                              kind="ExternalInput", name="ohx", uniquify=False)
            wcomb_d = dram.tile([P, T * TBLK], dt.float8e4,
                                kind="ExternalInput", name="wcomb", uniquify=False)
            wo_d = dram.tile([P, 3 * 2 * 96], dt.float8e4,
                             kind="ExternalInput", name="w_o", uniquify=False)
            bo_d = dram.tile([P, 1], dt.float32, kind="ExternalInput",
                             name="bo_pad", uniquify=False)
            out_d = dram.tile([OUT, b_core], dt.bfloat16, kind="ExternalOutput",
                              name="out", uniquify=False)

            with (
                tc.tile_pool(name="const", bufs=1) as const,
                tc.tile_pool(name="wt", bufs=3) as wt_pool,
                tc.tile_pool(name="sq", bufs=3) as sq_pool,
                tc.tile_pool(name="h3", bufs=3) as h3_pool,
                tc.tile_pool(name="exq", bufs=3) as ex_pool,
                tc.tile_pool(name="hps", bufs=1, space="PSUM") as hps_pool,
                tc.tile_pool(name="ltps", bufs=2, space="PSUM") as ltps_pool,
            ):
                # widx first: the gathers' only real dependency (tracked by
                # the tile framework); HWDGE preloads run concurrently with
                # SWDGE gathers (verified clean on HW).
                widx_sb = const.tile([P, T * n_chunks * 32], dt.int16,
                                     name="widx_sb")
                nc.sync.dma_start(out=widx_sb[:], in_=widx_d[:])
                wcomb_sb = const.tile([P, T * TBLK], dt.float8e4, name="wcomb_sb")
                nc.sync.dma_start(out=wcomb_sb[:], in_=wcomb_d[:])
                wo_sb = const.tile([P, 3 * 2 * 96], dt.float8e4, name="wo_sb")
                nc.sync.dma_start(out=wo_sb[:], in_=wo_d[:])
                bo_sb = const.tile([P, 1], dt.float32, name="bo_sb")
                nc.sync.dma_start(out=bo_sb[:], in_=bo_d[:])

                wo_v = wo_sb.rearrange("p (r j m) -> p r j m", r=3, j=2)

                def wslice(t, mi, msz):
                    o = t * TBLK + MOFF[mi]
                    return wcomb_sb[:, o:o + 2 * msz].rearrange(
                        "p (j m) -> p j m", j=2)

                # zero the h3 tail (plane 5, partitions 60..127) once per
                # physical buffer: stale SBUF bytes there could decode as
                # fp8 NaN and poison the logits accumulation.
                h3_warm = []
                for _ in range(3):
                    hw = h3_pool.tile([P, 3, 2, CHUNK], dt.float8e4, name="h3")
                    nc.vector.memset(hw[:, 2, 1, :], 0.0)
                    h3_warm.append(hw)

                pend = {}

                def emit_logits_pair(h3_ref, pr):
                    if pr == 0:
                        pend["lg"] = ltps_pool.tile([P, CHUNK], dt.float32,
                                                    name="lg", tag="lt")
                    nc.tensor.matmul(
                        pend["lg"][:96, :], wo_v[:, pr, :, :],
                        h3_ref[:, pr, :, :],
                        start=(pr == 0), stop=(pr == 2), perf_mode=DR)

                def emit_ex():
                    ex = ex_pool.tile([96, CHUNK], dt.bfloat16, name="ex")
                    nc.scalar.activation(ex[:96, :], pend["lg"][:96, :],
                                         mybir.ActivationFunctionType.Exp,
                                         bias=bo_sb[:96, :], scale=1.0 / 4096)
                    pend["ex"] = ex

                def emit_out(cc):
                    nc.sync.dma_start(out=out_d[:, cc * CHUNK:(cc + 1) * CHUNK],
                                      in_=pend["ex"][:OUT, :])

                qn = 0
                prev_h3 = None
                for c in range(n_chunks):
                    wt = wt_pool.tile([P, T, CHUNK], dt.uint16, name="wt")
                    for t in range(T):
                        nc.gpsimd.dma_gather(
                            wt[:, t:t + 1, :], word_tab[:],
                            widx_sb[:, (t * n_chunks + c) * 32:
                                    (t * n_chunks + c + 1) * 32],
                            CHUNK, CHUNK, 128, transpose=True,
                            queue_num=qn % NQ,
                        )
                        qn += 1
                    # host one-hots (pos/dep/bias) overwrite partitions
                    # 50..98 of every token's gathered tile
                    nc.sync.dma_start(
                        out=wt[50:99, :, :].rearrange("p a b -> p (a b)"),
                        in_=ohx_d[:, c * T * CHUNK:(c + 1) * T * CHUNK])

                    h3t = h3_pool.tile([P, 3, 2, CHUNK], dt.float8e4, name="h3")
                    for mi, (m0, msz) in enumerate(MT):
                        hp = hps_pool.tile([P, CHUNK], dt.float32,
                                           name=f"hps{mi}")
                        for t in range(T):
                            rhs = wt[:, t, :].bitcast(dt.float8e4).rearrange(
                                "p (n j) -> p j n", j=2)
                            nc.tensor.matmul(
                                hp[:msz, :], wslice(t, mi, msz),
                                rhs, start=(t == 0), stop=(t == T - 1),
                                perf_mode=DR)
                        sq = sq_pool.tile([P, CHUNK], dt.float32, name="sq")
                        nc.scalar.activation(sq[:msz, :], hp[:msz, :],
                                             mybir.ActivationFunctionType.Square,
                                             scale=1.0 / 256)
                        nc.vector.tensor_mul(h3t[:msz, mi // 2, mi % 2, :],
                                             sq[:msz, :], hp[:msz, :])
                        if mi == 0 and prev_h3 is not None:
                            emit_logits_pair(prev_h3, 2)
                        elif mi == 1 and prev_h3 is not None:
                            emit_ex()
                        elif mi == 2 and prev_h3 is not None:
                            emit_out(c - 1)
                        elif mi == 3:
                            emit_logits_pair(h3t, 0)
                        elif mi == 4:
                            emit_logits_pair(h3t, 1)
                    prev_h3 = h3t

                emit_logits_pair(prev_h3, 2)
                emit_ex()
                emit_out(n_chunks - 1)
    nc.compile()
    return nc


def _wrap_idx(idx_tc):
    """[CHUNK] -> [128, 32] wrapped (i -> [i%16, i//16]) + replicated x8."""
    n = idx_tc.shape[0]
    w = idx_tc.reshape(n // 16, 16).T
    return np.tile(w, (8, 1))


def prep_inputs(word_idx, pos_idx, dep_idx, word_table, pos_table, dep_table,
                Ww, bw, Wp, bp, Wd, bd, Wo, bo, b_core):
    """Returns (shared_map, per_core_fn)."""
    n_chunks = b_core // CHUNK

    # ---- word table: 256B rows of fp8 pairs (x16) ----
    wt8 = np.zeros((V + 1, 256), dtype=np.uint8)
    wt8[:V, :D] = (np.asarray(word_table, np.float32) * 16).astype(f8).view(
        np.uint8)
    wt_q = wt8.view(np.uint16)  # [V+1, 128]

    # ---- combined weights [128, T, 2, H] fp8 ----
    Ww32 = np.asarray(Ww, np.float32)
    Wp32 = np.asarray(Wp, np.float32)
    Wd32 = np.asarray(Wd, np.float32)
    pt32 = np.asarray(pos_table, np.float32)
    dt32 = np.asarray(dep_table, np.float32)
    bias_all = (np.asarray(bw, np.float32) + np.asarray(bp, np.float32)
                + np.asarray(bd, np.float32))

    wfull = np.zeros((P, T, 2, H), dtype=f8)
    for t in range(T):
        wq = (Ww32[D * t:D * (t + 1)] * 16).astype(f8)        # [100, H]
        f = np.arange(D)
        wfull[f // 2, t, f % 2, :] = wq
        projp = (pt32 @ Wp32[D * t:D * (t + 1)] * 256).astype(f8)  # [50, H]
        s = np.arange(NPOS)
        wfull[50 + s // 2, t, s % 2, :] = projp
        projd = (dt32 @ Wd32[D * t:D * (t + 1)] * 256).astype(f8)  # [45, H]
        s = np.arange(NDEP)
        wfull[75 + s // 2, t, s % 2, :] = projd
    wfull[98, 0, 0, :] = (bias_all * 256).astype(f8)
    # repack into contiguous per-(t, M-tile) [2, msz] blocks
    wcomb_host = np.zeros((P, T * TBLK), dtype=f8)
    for t in range(T):
        for mi, (m0, msz) in enumerate(MT):
            mhi = min(m0 + msz, H)
            blk = np.zeros((P, 2, msz), dtype=f8)
            blk[:, :, :mhi - m0] = wfull[:, t, :, m0:mhi]
            o = t * TBLK + MOFF[mi]
            wcomb_host[:, o:o + 2 * msz] = blk.reshape(P, 2 * msz)

    # ---- logits weights [128, 3, 2, 96] fp8: slot (p,pr,j) = h-feat
    # 128*(2pr+j)+p ----
    Wo32 = np.asarray(Wo, np.float32)
    wo = np.zeros((P, 3, 2, 96), dtype=f8)
    for pr in range(3):
        for j in range(2):
            mi = 2 * pr + j
            m0, msz = MT[mi]
            mhi = min(m0 + msz, H)
            wo[:mhi - m0, pr, j, :OUT] = (Wo32[m0:mhi] * 16).astype(f8)
    wo_host = np.ascontiguousarray(wo).reshape(P, 3 * 2 * 96)

    bo_pad = np.zeros((P, 1), dtype=np.float32)
    bo_pad[:OUT, 0] = np.asarray(bo, np.float32)

    shared = {
        "word_tab": wt_q,
        "wcomb": wcomb_host,
        "w_o": wo_host,
        "bo_pad": bo_pad,
    }

    wi = np.asarray(word_idx, np.int64).copy()
    wi[wi < 0] = V
    wi = wi.astype(np.int16)
    pi = np.asarray(pos_idx, np.int32)
    di = np.asarray(dep_idx, np.int32)

    def core_map(core):
        s = slice(core * b_core, (core + 1) * b_core)
        wic = wi[s]
        widx = np.zeros((P, T, n_chunks, 32), dtype=np.int16)
        for t in range(T):
            for c in range(n_chunks):
                widx[:, t, c, :] = _wrap_idx(wic[c * CHUNK:(c + 1) * CHUNK, t])

        # one-hot pairs: [49, 2, n_chunks, T, CHUNK] u8 -> u16 [49, ...]
        oh = np.zeros((49, 2, n_chunks, T, CHUNK), dtype=np.uint8)
        pc = pi[s].reshape(n_chunks, CHUNK, T)
        dc = di[s].reshape(n_chunks, CHUNK, T)
        cg, ig, tg = np.meshgrid(np.arange(n_chunks), np.arange(CHUNK),
                                 np.arange(T), indexing="ij")
        oh[pc // 2, pc % 2, cg, tg, ig] = F8_ONE
        oh[25 + dc // 2, dc % 2, cg, tg, ig] = F8_ONE
        oh[48, 0, :, 0, :] = F8_ONE  # bias rides token 0
        ohx = np.ascontiguousarray(oh.transpose(0, 2, 3, 4, 1)).view(
            np.uint16).reshape(49, n_chunks * T * CHUNK)

        m = dict(shared)
        m["widx"] = widx.reshape(P, T * n_chunks * 32)
        m["ohx"] = ohx
        return m

    return shared, core_map


def kernel(**inputs):
    b_core = B_CORE
    if b_core not in _NC_CACHE:
        _NC_CACHE[b_core] = build_nc(b_core)
    nc = _NC_CACHE[b_core]

    _, core_map = prep_inputs(b_core=b_core, **inputs)
    in_maps = [core_map(i) for i in range(NCORES)]
    res = run_bass_kernel_spmd(nc, in_maps, core_ids=list(range(NCORES)))
    ex = np.concatenate([r["out"] for r in res.results], axis=1)  # [93, B]
    ex = np.ascontiguousarray(ex.T).astype(np.float32)            # [B, 93]
    return ex / ex.sum(axis=1, keepdims=True)


# revision 16
# speedup vs baseline: 2.3711x; 1.0260x over previous
"""Trainium2 Bass kernel for nn_DependencyParsing (embedding_lookup).

Strategy (pure data-parallel over 8 NeuronCores, B=65536 -> 8192/core):
  - Everything on the PE runs as fp8(e4m3) DoubleRow matmuls: each
    instruction contracts 256 K-slots (128 partitions x 2 byte-planes)
    at the same per-instruction cost as bf16 (~226ns @ N=512).
  - word_table rows are 256B of packed fp8 feature-pairs (x16 scale):
    feature f of a row lands at SBUF (partition f//2, byte f%2) via the
    same u16 transpose dma_gather as a bf16 table would use. 4 SWDGE
    queues sustain ~1.27us/gather (112 gathers -> ~142us), overlapped
    with compute. Gathers also zero partitions 50..127 (table row tail).
  - pos/dep one-hots are precomputed on the host as fp8 pairs and DMA'd
    into partitions 50..98 of the *same* gathered tiles (after the
    gathers; HWDGE concurrent with SWDGE verified clean). The matching
    rows of the combined weight tensor hold host-computed
    proj = table_s @ W_s (x256) and the summed bias (x256) rides a
    constant-one slot at (t=0, partition 98). So h = x @ W is just
    7 DoubleRow matmuls per M-tile: 42 + 3 (logits) = 45 PE instrs per
    512-sample chunk.
  - h (PSUM, x256) -> ACT Square(scale 2^-8) -> DVE mul -> h3 fp8 (x256)
    packed [128, 3, 2, 512] for 3 DoubleRow logits matmuls (Wo x16).
  - ACT Exp(lg x 2^-12 + bo) -> ex bf16 [93, 512] -> DMA out per chunk.
    Softmax normalization (divide by row sum) happens on the host.
  - The previous chunk's logits/exp/out are interleaved between the
    current chunk's M-tiles so the PE never idles long enough for the
    HAM clock gate to re-throttle.
"""

import os

import numpy as np
import ml_dtypes

import concourse.bacc as bacc
import concourse.mybir as mybir
import concourse.tile as tile
from concourse.tile import add_dep_helper
from concourse.bass_utils import run_bass_kernel_spmd

B, T, D, H, V, NPOS, NDEP, OUT = 65536, 7, 100, 700, 32000, 50, 45, 93
NCORES = 8
B_CORE = B // NCORES
CHUNK = 512
P = 128
MT = [(0, 128), (128, 128), (256, 128), (384, 128), (512, 128), (640, 64)]
MOFF = [0, 256, 512, 768, 1024, 1280]  # 2*msz-prefix offsets within a t-block
TBLK = 1408  # 2 * sum(msz) per token
dt = mybir.dt
f8 = ml_dtypes.float8_e4m3
bf16 = ml_dtypes.bfloat16
NQ = int(os.environ.get("KERNEL_NQ", "4"))
DR = mybir.MatmulPerfMode.DoubleRow
F8_ONE = np.float32(1.0).astype(f8).view(np.uint8)  # 0x38

_NC_CACHE = {}


def build_nc(b_core):
    n_chunks = b_core // CHUNK
    nc = bacc.Bacc(None, target_bir_lowering=False, num_swdge_queues=NQ)
    with tile.TileContext(nc) as tc:
        with tc.tile_pool(name="dram", bufs=1, space="DRAM") as dram:
            word_tab = dram.tile([V + 1, 128], dt.uint16, kind="ExternalInput",
                                 name="word_tab", uniquify=False)
            widx_d = dram.tile([P, T * n_chunks * 32], dt.int16,
                               kind="ExternalInput", name="widx", uniquify=False)
            ohx_d = dram.tile([49, n_chunks * T * CHUNK], dt.uint16,
                              kind="ExternalInput", name="ohx", uniquify=False)
            wcomb_d = dram.tile([P, T * TBLK], dt.float8e4,
                                kind="ExternalInput", name="wcomb", uniquify=False)
            wo_d = dram.tile([P, 3 * 2 * 96], dt.float8e4,
                             kind="ExternalInput", name="w_o", uniquify=False)
            bo_d = dram.tile([P, 1], dt.float32, kind="ExternalInput",
                             name="bo_pad", uniquify=False)
            out_d = dram.tile([OUT, b_core], dt.bfloat16, kind="ExternalOutput",
                              name="out", uniquify=False)

            with (
                tc.tile_pool(name="const", bufs=1) as const,
                tc.tile_pool(name="wt", bufs=3) as wt_pool,
                tc.tile_pool(name="sq", bufs=3) as sq_pool,
                tc.tile_pool(name="h3", bufs=3) as h3_pool,
                tc.tile_pool(name="exq", bufs=3) as ex_pool,
                tc.tile_pool(name="hps", bufs=1, space="PSUM") as hps_pool,
                tc.tile_pool(name="ltps", bufs=2, space="PSUM") as ltps_pool,
            ):
                # widx is chunk-major and preloaded in per-chunk pieces so
                # chunk 0's gathers only wait on a 57KB DMA, not the whole
                # 0.9MB. HWDGE preloads run concurrently with SWDGE gathers
                # (verified clean on HW).
                widx_sb = const.tile([P, n_chunks * T * 32], dt.int16,
                                     name="widx_sb")
                wseg = T * 32

                def widx_piece(c):
                    nc.sync.dma_start(
                        out=widx_sb[:, c * wseg:(c + 1) * wseg],
                        in_=widx_d[:, c * wseg:(c + 1) * wseg])

                widx_piece(0)
                widx_piece(1)
                wcomb_sb = const.tile([P, T * TBLK], dt.float8e4, name="wcomb_sb")
                nc.sync.dma_start(out=wcomb_sb[:], in_=wcomb_d[:])
                widx_piece(2)
                widx_piece(3)
                wo_sb = const.tile([P, 3 * 2 * 96], dt.float8e4, name="wo_sb")
                nc.sync.dma_start(out=wo_sb[:], in_=wo_d[:])
                bo_sb = const.tile([P, 1], dt.float32, name="bo_sb")
                nc.sync.dma_start(out=bo_sb[:], in_=bo_d[:])
                for c in range(4, n_chunks):
                    widx_piece(c)

                wo_v = wo_sb.rearrange("p (r j m) -> p r j m", r=3, j=2)

                def wslice(t, mi, msz):
                    o = t * TBLK + MOFF[mi]
                    return wcomb_sb[:, o:o + 2 * msz].rearrange(
                        "p (j m) -> p j m", j=2)

                # zero the h3 tail (plane 5, partitions 60..127) once per
                # physical buffer: stale SBUF bytes there could decode as
                # fp8 NaN and poison the logits accumulation.
                h3_warm = []
                for _ in range(3):
                    hw = h3_pool.tile([P, 3, 2, CHUNK], dt.float8e4, name="h3")
                    nc.vector.memset(hw[:, 2, 1, :], 0.0)
                    h3_warm.append(hw)

                pend = {}

                def emit_logits_pair(h3_ref, pr):
                    if pr == 0:
                        pend["lg"] = ltps_pool.tile([P, CHUNK], dt.float32,
                                                    name="lg", tag="lt")
                    nc.tensor.matmul(
                        pend["lg"][:96, :], wo_v[:, pr, :, :],
                        h3_ref[:, pr, :, :],
                        start=(pr == 0), stop=(pr == 2), perf_mode=DR)

                def emit_ex():
                    ex = ex_pool.tile([96, CHUNK], dt.bfloat16, name="ex")
                    nc.scalar.activation(ex[:96, :], pend["lg"][:96, :],
                                         mybir.ActivationFunctionType.Exp,
                                         bias=bo_sb[:96, :], scale=1.0 / 4096)
                    pend["ex"] = ex

                def emit_out(cc):
                    nc.sync.dma_start(out=out_d[:, cc * CHUNK:(cc + 1) * CHUNK],
                                      in_=pend["ex"][:OUT, :])

                qn = 0
                prev_h3 = None
                for c in range(n_chunks):
                    wt = wt_pool.tile([P, T, CHUNK], dt.uint16, name="wt")
                    for t in range(T):
                        nc.gpsimd.dma_gather(
                            wt[:, t:t + 1, :], word_tab[:],
                            widx_sb[:, (c * T + t) * 32:
                                    (c * T + t + 1) * 32],
                            CHUNK, CHUNK, 128, transpose=True,
                            queue_num=qn % NQ,
                        )
                        qn += 1
                    # host one-hots (pos/dep/bias) overwrite partitions
                    # 50..98 of every token's gathered tile
                    nc.sync.dma_start(
                        out=wt[50:99, :, :].rearrange("p a b -> p (a b)"),
                        in_=ohx_d[:, c * T * CHUNK:(c + 1) * T * CHUNK])

                    h3t = h3_pool.tile([P, 3, 2, CHUNK], dt.float8e4, name="h3")
                    for mi, (m0, msz) in enumerate(MT):
                        hp = hps_pool.tile([P, CHUNK], dt.float32,
                                           name=f"hps{mi}")
                        for t in range(T):
                            rhs = wt[:, t, :].bitcast(dt.float8e4).rearrange(
                                "p (n j) -> p j n", j=2)
                            nc.tensor.matmul(
                                hp[:msz, :], wslice(t, mi, msz),
                                rhs, start=(t == 0), stop=(t == T - 1),
                                perf_mode=DR)
                        sq = sq_pool.tile([P, CHUNK], dt.float32, name="sq")
                        nc.scalar.activation(sq[:msz, :], hp[:msz, :],
                                             mybir.ActivationFunctionType.Square,
                                             scale=1.0 / 256)
                        nc.vector.tensor_mul(h3t[:msz, mi // 2, mi % 2, :],
                                             sq[:msz, :], hp[:msz, :])
                        if mi == 0 and prev_h3 is not None:
                            emit_logits_pair(prev_h3, 2)
                        elif mi == 1 and prev_h3 is not None:
                            emit_ex()
                        elif mi == 2 and prev_h3 is not None:
                            emit_out(c - 1)
                        elif mi == 3:
                            emit_logits_pair(h3t, 0)
                        elif mi == 4:
                            emit_logits_pair(h3t, 1)
                    prev_h3 = h3t

                emit_logits_pair(prev_h3, 2)
                emit_ex()
                emit_out(n_chunks - 1)
    nc.compile()
    return nc


def _wrap_idx(idx_tc):
    """[CHUNK] -> [128, 32] wrapped (i -> [i%16, i//16]) + replicated x8."""
    n = idx_tc.shape[0]
    w = idx_tc.reshape(n // 16, 16).T
    return np.tile(w, (8, 1))


def prep_inputs(word_idx, pos_idx, dep_idx, word_table, pos_table, dep_table,
                Ww, bw, Wp, bp, Wd, bd, Wo, bo, b_core):
    """Returns (shared_map, per_core_fn)."""
    n_chunks = b_core // CHUNK

    # ---- word table: 256B rows of fp8 pairs (x16) ----
    wt8 = np.zeros((V + 1, 256), dtype=np.uint8)
    wt8[:V, :D] = (np.asarray(word_table, np.float32) * 16).astype(f8).view(
        np.uint8)
    wt_q = wt8.view(np.uint16)  # [V+1, 128]

    # ---- combined weights [128, T, 2, H] fp8 ----
    Ww32 = np.asarray(Ww, np.float32)
    Wp32 = np.asarray(Wp, np.float32)
    Wd32 = np.asarray(Wd, np.float32)
    pt32 = np.asarray(pos_table, np.float32)
    dt32 = np.asarray(dep_table, np.float32)
    bias_all = (np.asarray(bw, np.float32) + np.asarray(bp, np.float32)
                + np.asarray(bd, np.float32))

    wfull = np.zeros((P, T, 2, H), dtype=f8)
    for t in range(T):
        wq = (Ww32[D * t:D * (t + 1)] * 16).astype(f8)        # [100, H]
        f = np.arange(D)
        wfull[f // 2, t, f % 2, :] = wq
        projp = (pt32 @ Wp32[D * t:D * (t + 1)] * 256).astype(f8)  # [50, H]
        s = np.arange(NPOS)
        wfull[50 + s // 2, t, s % 2, :] = projp
        projd = (dt32 @ Wd32[D * t:D * (t + 1)] * 256).astype(f8)  # [45, H]
        s = np.arange(NDEP)
        wfull[75 + s // 2, t, s % 2, :] = projd
    wfull[98, 0, 0, :] = (bias_all * 256).astype(f8)
    # repack into contiguous per-(t, M-tile) [2, msz] blocks
    wcomb_host = np.zeros((P, T * TBLK), dtype=f8)
    for t in range(T):
        for mi, (m0, msz) in enumerate(MT):
            mhi = min(m0 + msz, H)
            blk = np.zeros((P, 2, msz), dtype=f8)
            blk[:, :, :mhi - m0] = wfull[:, t, :, m0:mhi]
            o = t * TBLK + MOFF[mi]
            wcomb_host[:, o:o + 2 * msz] = blk.reshape(P, 2 * msz)

    # ---- logits weights [128, 3, 2, 96] fp8: slot (p,pr,j) = h-feat
    # 128*(2pr+j)+p ----
    Wo32 = np.asarray(Wo, np.float32)
    wo = np.zeros((P, 3, 2, 96), dtype=f8)
    for pr in range(3):
        for j in range(2):
            mi = 2 * pr + j
            m0, msz = MT[mi]
            mhi = min(m0 + msz, H)
            wo[:mhi - m0, pr, j, :OUT] = (Wo32[m0:mhi] * 16).astype(f8)
    wo_host = np.ascontiguousarray(wo).reshape(P, 3 * 2 * 96)

    bo_pad = np.zeros((P, 1), dtype=np.float32)
    bo_pad[:OUT, 0] = np.asarray(bo, np.float32)

    shared = {
        "word_tab": wt_q,
        "wcomb": wcomb_host,
        "w_o": wo_host,
        "bo_pad": bo_pad,
    }

    wi = np.asarray(word_idx, np.int64).copy()
    wi[wi < 0] = V
    wi = wi.astype(np.int16)
    pi = np.asarray(pos_idx, np.int32)
    di = np.asarray(dep_idx, np.int32)

    def core_map(core):
        s = slice(core * b_core, (core + 1) * b_core)
        wic = wi[s]
        widx = np.zeros((P, n_chunks, T, 32), dtype=np.int16)
        for t in range(T):
            for c in range(n_chunks):
                widx[:, c, t, :] = _wrap_idx(wic[c * CHUNK:(c + 1) * CHUNK, t])

        # one-hot pairs: [49, 2, n_chunks, T, CHUNK] u8 -> u16 [49, ...]
        oh = np.zeros((49, 2, n_chunks, T, CHUNK), dtype=np.uint8)
        pc = pi[s].reshape(n_chunks, CHUNK, T)
        dc = di[s].reshape(n_chunks, CHUNK, T)
        cg, ig, tg = np.meshgrid(np.arange(n_chunks), np.arange(CHUNK),
                                 np.arange(T), indexing="ij")
        oh[pc // 2, pc % 2, cg, tg, ig] = F8_ONE
        oh[25 + dc // 2, dc % 2, cg, tg, ig] = F8_ONE
        oh[48, 0, :, 0, :] = F8_ONE  # bias rides token 0
        ohx = np.ascontiguousarray(oh.transpose(0, 2, 3, 4, 1)).view(
            np.uint16).reshape(49, n_chunks * T * CHUNK)

        m = dict(shared)
        m["widx"] = widx.reshape(P, T * n_chunks * 32)
        m["ohx"] = ohx
        return m

    return shared, core_map


def kernel(**inputs):
    b_core = B_CORE
    if b_core not in _NC_CACHE:
        _NC_CACHE[b_core] = build_nc(b_core)
    nc = _NC_CACHE[b_core]

    _, core_map = prep_inputs(b_core=b_core, **inputs)
    in_maps = [core_map(i) for i in range(NCORES)]
    res = run_bass_kernel_spmd(nc, in_maps, core_ids=list(range(NCORES)))
    ex = np.concatenate([r["out"] for r in res.results], axis=1)  # [93, B]
    ex = np.ascontiguousarray(ex.T).astype(np.float32)            # [B, 93]
    return ex / ex.sum(axis=1, keepdims=True)
